# revision 26
# baseline (speedup 1.0000x reference)
"""Trainium2 Bass kernel for nn_EulerIntegratorCell (Euler-integration RNN).

Reference computation (per batch row b, sequentially over t = 0..T-1):
    z_t  = concat(x_t, a_{t-1}) @ W1 + b1        (HID=64)
    dk_t = tanh(z_t) @ W2 + b2                   (> 0)
    a_t  = a_{t-1} + C * dk_t ** M               (C=1.5e-11, M=3.8)

Kernel strategy
---------------
1. Linearize the per-step increment g(x, a) = C*dk**M in `a` around each
   row's initial state a0 (drift <= 7e-3 over all T=2048 steps; truncation
   ~1e-8), giving a linear recurrence directly on a_t:
       a_t = d0(a0) * a_{t-1} + d1_t,   d1_t = A(a0)*sigmoid(2*x_t + b(a0))
                                              + T0'(a0)
   with d0 = 1 + Gmean, T0' = T0 - Gmean*a0. This maps onto the DVE
   `tensor_tensor_scan` instruction (fp32 internal state, per-partition
   initial value = a0, output downcast to bf16 at write).

2. The sigmoid surrogate for g(x, a0) is fit host-side from the passed-in
   weights (grid lstsq; max residual ~1.9e-8 of g~4e-6). The per-row
   coefficients (b, A, T0', d0, a0) are evaluated host-side (O(B) work)
   and uploaded as a tiny [128, 5*NT] table -- no device-side Horner.

3. Dataflow per [128 x 2048] tile: DMA-in x as uint8 (exact dequant via
   the ACT input scale 2/255) -> ACT sigmoid (bf16) -> per-row affine
   d1 = A*sg + T0' (bf16; split across Pool/ACT/DVE to balance engine
   load) -> DVE scan (fp32 state, bf16 out, initial=a0) -> DMA-out bf16.
   Scheduling: sigma of the first Pool tile leads the ACT stream (Pool is
   the scarcest engine; its start is sigma-gated), tile 0 is processed in
   four chained column chunks (fp32 carry, separate fp32 output tensor)
   so the DVE scan pipeline starts ~4us in, input DMAs
   are batched up-front on SP, and out-DMAs ride SP with an emission lag
   so their semaphore waits never block a busy sequencer (DMA waits hold
   the issuing SEQ).

4. Data-parallel over 8 NeuronCores: batch 16384 -> 2048 rows per core,
   no cross-core communication.

End-to-end relative error vs the fp32 reference: ~4.4e-3 (gate 2e-2),
dominated by the bf16 output rounding (terminal, non-accumulating).
"""

import numpy as np
from contextlib import ExitStack

# Problem constants (hardcoded per harness contract).
C = 1.5e-11
M = 3.8
B, T, HID = 16384, 2048, 64
N_CORES = 8
B_CORE = B // N_CORES          # 2048 rows per core
NT = B_CORE // 128             # 16 batch tiles of 128 rows per core
ADEG = 12                      # degree of the a0-polynomials
NFUNC = 4                      # T0, A, b, Gmean
EXP_C = 2.0                    # sigmoid steepness (global)
NCOEF = 5                      # b, A, T0', d0, a0 per-row device table

# Which engine computes d1 = A*sg + T0' for tile i (balance engine busy):
#   'P' Pool (~2.9us/tile), 'A' ACT (~1.9us), 'V' DVE (~0.6us bf16 4x).
# DVE takes the first two (its scan pipeline is still filling then).
D1_ENGINE = "VPPVPPAPPVPPAPPP"
OUT_LAG = 3          # emit out-DMA i after scan i+OUT_LAG so its semaphore
                     # wait is satisfied at decode (DMA waits hold the SEQ)
SPLIT0 = 4           # process tile 0 in SPLIT0 column chunks (chained scans,
                     # f32 carry) so the DVE pipeline starts ~3us earlier
SPLITL = 0           # likewise chunk the last tile (0 = disabled: the tail
                     # is Pool-bound, so chunking it bought nothing)
ACT_D1_LAG = 2       # emit an ACT-engine d1 this many sigmas later so it
                     # does not stall the sigma stream


def _fit_params(W1, b1, W2, b2):
    """Host-side fit of the sigmoid surrogate (O(grid) work, ~2s).

    Returns PC[4, ADEG+1]: power-basis coefficients in t = 2*a0 - 1 for
    (T0, A, b, Gmean)."""
    from scipy.optimize import minimize_scalar
    W1 = np.asarray(W1, np.float64)
    b1 = np.asarray(b1, np.float64)
    W2 = np.asarray(W2, np.float64).reshape(-1)
    b2v = float(np.asarray(b2).reshape(-1)[0])
    al, be, ga = W1[0], W1[1], b1
    NX, NA = 513, 257
    xs = np.linspace(0.0, 1.0, NX)
    as_ = np.linspace(0.0, 1.0, NA)
    z = xs[:, None, None] * al + as_[None, :, None] * be + ga
    th = np.tanh(z)
    dk = th @ W2 + b2v
    G = C * dk ** M
    GA = C * M * dk ** (M - 1.0) * ((1.0 - th * th) @ (W2 * be))
    sig = lambda v: 1.0 / (1.0 + np.exp(-v))
    T0v = np.empty(NA); Av = np.empty(NA); bv = np.empty(NA)
    for ia in range(NA):
        g = G[:, ia]
        def err_b(b):
            Phi = np.stack([np.ones(NX), sig(EXP_C * xs + b)], 1)
            sol, *_ = np.linalg.lstsq(Phi, g, rcond=None)
            return np.abs(Phi @ sol - g).max()
        res = minimize_scalar(err_b, bounds=(-6.0, 4.0), method="bounded",
                              options={"xatol": 1e-10})
        Phi = np.stack([np.ones(NX), sig(EXP_C * xs + res.x)], 1)
        sol, *_ = np.linalg.lstsq(Phi, g, rcond=None)
        T0v[ia], Av[ia] = sol
        bv[ia] = res.x
    funcs = np.stack([T0v, Av, bv, GA.mean(axis=0)])
    cc = np.polynomial.chebyshev.chebfit(2 * as_ - 1, funcs.T, ADEG)
    rows = []
    for r in range(NFUNC):
        p = np.polynomial.chebyshev.cheb2poly(cc[:, r])
        rows.append(np.pad(p, (0, ADEG + 1 - len(p))))
    return np.array(rows)                                      # [4, ADEG+1]


def _build_nc():
    """Build + compile the per-core Bass program (identical on all cores)."""
    import concourse.tile as tile
    from concourse import bacc, mybir

    f32 = mybir.dt.float32
    bf16 = mybir.dt.bfloat16
    u8 = mybir.dt.uint8
    AF = mybir.ActivationFunctionType
    OP = mybir.AluOpType

    nc = bacc.Bacc("TRN2", target_bir_lowering=False, debug=False)
    xin = nc.dram_tensor("x_sh", [B_CORE, T], u8, kind="ExternalInput")
    cin = nc.dram_tensor("coef", [128, NCOEF * NT], f32, kind="ExternalInput")
    t0r = nc.dram_tensor("t0row", [1, NT * 128], f32, kind="ExternalInput")
    out = nc.dram_tensor("out_sh", [B_CORE, T], bf16, kind="ExternalOutput")
    out0 = nc.dram_tensor("out0_f32", [128, T], f32, kind="ExternalOutput")
    outL = nc.dram_tensor("outL_f32", [128, T], f32, kind="ExternalOutput")

    with tile.TileContext(nc) as tc, ExitStack() as ctx:
        cpool = ctx.enter_context(tc.tile_pool(name="consts", bufs=1))
        xpool = ctx.enter_context(tc.tile_pool(name="x", bufs=1))
        spool = ctx.enter_context(tc.tile_pool(name="sg", bufs=6))
        dpool = ctx.enter_context(tc.tile_pool(name="d1", bufs=6))
        opool = ctx.enter_context(tc.tile_pool(name="o", bufs=OUT_LAG + 3))
        import concourse.bass as cbass
        ppool = ctx.enter_context(
            tc.tile_pool(name="ps", bufs=1, space=cbass.MemorySpace.PSUM))

        # Dummy activation on a memset tile: forces the one-time
        # LoadActFuncSet (~1.3us) at t~0 instead of behind the first
        # input DMA.
        dum = cpool.tile([128, 1], f32)
        nc.vector.memset(dum[:], 0.0)
        dumo = cpool.tile([128, 1], bf16)
        nc.scalar.activation(dumo[:], dum[:], AF.Sigmoid, bias=0.0, scale=1.0)

        ct = cpool.tile([128, NCOEF * NT], f32)
        nc.sync.dma_start(ct[:], cin.ap())

        def col(f, i):
            return ct[:, f * NT + i: f * NT + i + 1]

        f32r = mybir.dt.float32r
        e_tiles = [i for i in range(NT) if D1_ENGINE[i] == "E"]
        if e_tiles:
            t0row = cpool.tile([1, NT * 128], f32)
            nc.sync.dma_start(t0row[:], t0r.ap())
            ones = cpool.tile([1, 512], f32)
            nc.gpsimd.memset(ones[:], 1.0)
            io_f = cpool.tile([128, 128], mybir.dt.int32)
            nc.gpsimd.iota(io_f[:], [[1, 128]], base=0, channel_multiplier=0)
            io_p = cpool.tile([128, 128], mybir.dt.int32)
            nc.gpsimd.iota(io_p[:], [[0, 128]], base=0, channel_multiplier=1)
            mask = cpool.tile([128, 128], f32)
            nc.gpsimd.tensor_tensor(mask[:], io_f[:], io_p[:], OP.is_equal)
            diags = {}
            for i in e_tiles:
                dg = cpool.tile([128, 128], f32, name=f"diag{i}")
                nc.gpsimd.tensor_scalar(dg[:], mask[:], col(1, i), None,
                                        OP.mult)
                diags[i] = dg

        # All input DMAs up-front on the SP queue: one buffer per tile means
        # no write-after-read waits, so the configs stream back-to-back.
        # Tiles 0-1 as singles (fast pipeline start), the rest batched into
        # one DMA per group (fewer configs -> SP queue clears early for the
        # output DMAs).
        xts = [None] * NT               # tile i -> (sbuf tile, col offset)
        for i in (1, 0):                # first_p (tile 1) first: its sigma
            xt = xpool.tile([128, T], u8, name=f"xt{i}")   # leads the stream
            nc.sync.dma_start(xt[:], xin[i * 128:(i + 1) * 128, :])
            xts[i] = (xt, 0)
        for lo, hi in ((2, 4), (4, 8), (8, 12), (12, 16)):
            n = hi - lo
            xb = xpool.tile([128, n * T], u8, name=f"xb{lo}")
            src = xin[lo * 128:hi * 128, :].rearrange(
                "(blk p) t -> p blk t", p=128)
            dst = xb[:].rearrange("p (blk t) -> p blk t", t=T)
            nc.sync.dma_start(dst, src)
            for k in range(n):
                xts[lo + k] = (xb, k * T)

        def xap(i, cs=None):
            t, off = xts[i]
            cs = cs or slice(0, T)
            return t[:, off + cs.start:off + cs.stop]

        ot0 = cpool.tile([128, T], f32)     # tiles 0 and NT-1 keep fp32
        otL = cpool.tile([128, T], f32)     # (exact chunk chaining); DMA'd
                                            # out separately
        ots = {}
        scan_done = []                      # tiles whose scan has been emitted

        def emit_out(j):
            # All outs on the SP queue: it is idle after the input burst, so
            # a not-yet-satisfied wait holds nothing up (DMA waits hold the
            # issuing SEQ; ACT must stay clear to keep the sigma stream
            # gapless).
            if j == 0:
                nc.sync.dma_start(out0.ap(), ot0[:])
            elif j in chunk_emitted:
                ots.pop(j, None)          # emitted per chunk above
            else:
                nc.sync.dma_start(out[j * 128:(j + 1) * 128, :],
                                  ots.pop(j)[:])

        def note_scan(i):
            scan_done.append(i)
            if len(scan_done) > OUT_LAG:
                emit_out(scan_done[-OUT_LAG - 1])

        first_p = ("x" + D1_ENGINE[1:]).find("P")   # first P excluding tile 0
        chunk_emitted = set()           # tiles whose outs went per-chunk
        TAIL_CHUNK = set()              # chunk-chained scans + per-chunk outs
                                        # so the final out-DMA only trails a
                                        # short last chunk

        def emit_d1_scan(i, sg):
            eng = D1_ENGINE[i]
            if eng == "E":
                ps = ppool.tile([128, T], f32, tag="ps")
                sgr = sg[:].bitcast(f32r)
                dgr = diags[i][:].bitcast(f32r)
                t0c = t0row[:, i * 128:(i + 1) * 128].bitcast(f32r)
                onr = ones[:].bitcast(f32r)
                for c in range(4):
                    cs = slice(c * 512, (c + 1) * 512)
                    nc.tensor.matmul(ps[:, cs], dgr, sgr[:, cs.start:cs.stop],
                                     start=True, stop=False)
                for c in range(4):
                    cs = slice(c * 512, (c + 1) * 512)
                    nc.tensor.matmul(ps[:, cs], t0c, onr,
                                     start=False, stop=True)
                ot = opool.tile([128, T], bf16, tag="ot")
                nc.vector.tensor_tensor_scan(
                    ot[:], col(3, i).broadcast_to((128, T)), ps[:],
                    col(4, i), OP.mult, OP.add)
                ots[i] = ot
                note_scan(i)
                return
            d1 = dpool.tile([128, T], bf16, tag="d1")
            if eng == "P":
                if False:
                    # Halved d1 + chunk-chained scan: Pool starts on the
                    # first half ~1.5us earlier and DVE consumes it at once.
                    H = T // 2
                    nc.gpsimd.tensor_scalar(d1[:, 0:H], sg[:, 0:H],
                                            col(1, i), col(2, i),
                                            OP.mult, OP.add)
                    nc.gpsimd.tensor_scalar(d1[:, H:T], sg[:, H:T],
                                            col(1, i), col(2, i),
                                            OP.mult, OP.add)
                else:
                    nc.gpsimd.tensor_scalar(d1[:], sg[:], col(1, i),
                                            col(2, i), OP.mult, OP.add)
            elif eng == "V":
                nc.vector.tensor_scalar(d1[:], sg[:], col(1, i), col(2, i),
                                        OP.mult, OP.add)
            else:
                nc.scalar.activation(d1[:], sg[:], AF.Identity,
                                     bias=col(2, i), scale=col(1, i))
            ot = opool.tile([128, T], bf16, tag="ot")
            nch = 2 if i in TAIL_CHUNK else 1
            if nch == 1:
                nc.vector.tensor_tensor_scan(
                    ot[:], col(3, i).broadcast_to((128, T)), d1[:],
                    col(4, i), OP.mult, OP.add)
            else:
                CW = T // nch
                for c in range(nch):
                    cs = slice(c * CW, (c + 1) * CW)
                    init = col(4, i) if c == 0 else ot[:, c * CW - 1:c * CW]
                    nc.vector.tensor_tensor_scan(
                        ot[:, cs], col(3, i).broadcast_to((128, CW)),
                        d1[:, cs], init, OP.mult, OP.add)
                    if i in TAIL_CHUNK:
                        nc.sync.dma_start(out[i * 128:(i + 1) * 128,
                                              cs.start:cs.stop], ot[:, cs])
            ots[i] = ot
            if i in TAIL_CHUNK:
                chunk_emitted.add(i)
                scan_done.append(i)     # outs already emitted per chunk
            else:
                note_scan(i)

        deferred = {}                       # pos-due -> (tile, sg)

        # Sigma of the first Pool tile goes FIRST: Pool is the scarcest d1
        # engine, and its start time is gated by its first sigma. Tile 0's
        # quarter-sigmas follow; the first-P scan is emitted after tile 0's
        # quarter-scans so it does not head-of-line block DVE.
        fp_sg = None
        if first_p >= 0 and SPLIT0 > 1:
            fp_sg = spool.tile([128, T], bf16, tag="sgfp")
            nc.scalar.activation(fp_sg[:], xap(first_p), AF.Sigmoid,
                                 bias=col(0, first_p),
                                 scale=float(EXP_C / 255.0))

        for i in range(NT):
            if i == first_p and fp_sg is not None:
                emit_d1_scan(i, fp_sg)
                continue
            sg = spool.tile([128, T],
                            f32 if D1_ENGINE[i] == "E" else bf16, tag="sg")
            if i == 0 and SPLIT0 > 1:
                # Chunked first tile: sigma/d1/scan per 512-col chunk so the
                # DVE pipeline starts as soon as the first chunk lands.
                d1 = dpool.tile([128, T], bf16, tag="d1")
                CW = T // SPLIT0
                q_eng = nc.gpsimd if D1_ENGINE[0] == "P" else nc.vector
                for c in range(SPLIT0):
                    cs = slice(c * CW, (c + 1) * CW)
                    nc.scalar.activation(sg[:, cs], xap(0, cs), AF.Sigmoid,
                                         bias=col(0, 0),
                                         scale=float(EXP_C / 255.0))
                    q_eng.tensor_scalar(d1[:, cs], sg[:, cs], col(1, 0),
                                        col(2, 0), OP.mult, OP.add)
                    init = col(4, 0) if c == 0 else ot0[:, c * CW - 1:c * CW]
                    nc.vector.tensor_tensor_scan(
                        ot0[:, cs], col(3, 0).broadcast_to((128, CW)),
                        d1[:, cs], init, OP.mult, OP.add)
                note_scan(0)
            elif i == NT - 1 and SPLITL > 1:
                # Chunked last tile: fp32 chunk-chained scans with the out-DMA
                # of each chunk fired immediately -> the final transfer only
                # trails the last (short) chunk.
                d1 = dpool.tile([128, T], bf16, tag="d1")
                CW = T // SPLITL
                for c in range(SPLITL):
                    cs = slice(c * CW, (c + 1) * CW)
                    nc.scalar.activation(sg[:, cs], xap(i, cs), AF.Sigmoid,
                                         bias=col(0, i),
                                         scale=float(EXP_C / 255.0))
                    nc.vector.tensor_scalar(d1[:, cs], sg[:, cs], col(1, i),
                                            col(2, i), OP.mult, OP.add)
                    init = col(4, i) if c == 0 else otL[:, c * CW - 1:c * CW]
                    nc.vector.tensor_tensor_scan(
                        otL[:, cs], col(3, i).broadcast_to((128, CW)),
                        d1[:, cs], init, OP.mult, OP.add)
                    nc.sync.dma_start(outL[:, cs.start:cs.stop], otL[:, cs])
                scan_done.append(i)     # out already emitted per chunk
            else:
                nc.scalar.activation(sg[:], xap(i), AF.Sigmoid,
                                     bias=col(0, i), scale=float(EXP_C / 255.0))
                if D1_ENGINE[i] in ("A", "E"):
                    deferred[i + ACT_D1_LAG] = (i, sg)
                else:
                    emit_d1_scan(i, sg)
            if i in deferred:
                emit_d1_scan(*deferred.pop(i))
        for pos in sorted(deferred):
            emit_d1_scan(*deferred.pop(pos))
        for j in scan_done[-OUT_LAG:] if OUT_LAG else []:
            emit_out(j)

    nc.compile()
    return nc


_NC_CACHE = {}


def kernel(x, a0, W1, b1, W2, b2):
    x = np.asarray(x, np.float32)
    a0 = np.asarray(a0, np.float32)
    assert x.shape == (B, T, 1) and a0.shape == (B, 1), (x.shape, a0.shape)

    PC = _fit_params(W1, b1, W2, b2)

    key = "v11"
    if key not in _NC_CACHE:
        _NC_CACHE[key] = _build_nc()
    nc = _NC_CACHE[key]

    # Per-row coefficients, evaluated host-side in f64 then cast.
    a064 = a0[:, 0].astype(np.float64)
    tch = 2.0 * a064 - 1.0
    T0r = np.polyval(PC[0][::-1], tch)
    Ar = np.polyval(PC[1][::-1], tch)
    br = np.polyval(PC[2][::-1], tch)
    Gmr = np.polyval(PC[3][::-1], tch)
    coefs = np.stack([br, Ar, T0r - Gmr * a064, 1.0 + Gmr, a064],
                     axis=0).astype(np.float32)               # [5, B]

    xq = np.round(x[:, :, 0] * np.float32(255.0)).astype(np.uint8)
    in_maps = []
    for cidx in range(N_CORES):
        sl = slice(cidx * B_CORE, (cidx + 1) * B_CORE)
        xs = np.ascontiguousarray(xq[sl])
        # coef[p, f*NT + i] = coefficient f of batch row (core_base + i*128 + p)
        cf = coefs[:, sl].reshape(NCOEF, NT, 128)
        cf = np.ascontiguousarray(
            np.transpose(cf, (2, 0, 1)).reshape(128, NCOEF * NT))
        t0rows = coefs[2, sl].reshape(NT * 128)[None, :].astype(np.float32)
        in_maps.append({"x_sh": xs, "coef": cf,
                        "t0row": np.ascontiguousarray(t0rows)})

    from concourse.bass_utils import run_bass_kernel_spmd
    # The axon-tunneled device occasionally reports
    # NRT_EXEC_UNIT_UNRECOVERABLE on the first dispatch after a fresh
    # process start; it self-recovers within ~1 min.  Retry defensively.
    import time
    last_exc = None
    for attempt in range(4):
        try:
            res = run_bass_kernel_spmd(nc, in_maps,
                                       core_ids=list(range(N_CORES)))
            break
        except Exception as exc:   # noqa: BLE001 — device-level flake
            last_exc = exc
            time.sleep(20.0 * (attempt + 1))
            if attempt >= 1:
                # Rebuild in case the compiled executable is poisoned.
                _NC_CACHE.pop(key, None)
                _NC_CACHE[key] = nc = _build_nc()
    else:
        raise last_exc
    blocks = []
    for cidx in range(N_CORES):
        blk = np.asarray(res.results[cidx]["out_sh"]).astype(np.float32)
        # Tile 0 (rows 0:128) travelled as fp32 via the separate out0_f32
        # tensor (chunk-chained scan); stitch it in.
        blk[0:128] = np.asarray(res.results[cidx]["out0_f32"])
        if SPLITL > 1:
            blk[B_CORE - 128:] = np.asarray(res.results[cidx]["outL_f32"])
        blocks.append(blk)
    out = np.concatenate(blocks, axis=0)
    return np.ascontiguousarray(out[:, :, None].astype(np.float32))
